# revision 1
# baseline (speedup 1.0000x reference)
"""Attentional pooling layer on Trainium2 (Bass/Tile), 8-core batch-parallel.

Reference computation per batch b:
    scores[hw, n] = sum_c f[c, hw] * w[c, n]          (mm1, fp32)
    num           = softplus(scores)                  (ACT: Abs/Exp/Ln)
    denom[n]      = sum_hw num[hw, n] + 16*CONST      (PE reduce + DVE)
    att[hw, n]    = (num + CONST) / denom[n]          (PE bcast + DVE)
    out[c, n]     = sum_hw f[c, hw] * att[hw, n]      (mm2, float32r)

Partition layout: 3 batches are packed into one 96-partition group at
32-partition offsets (PE tile_position only supports 32-aligned output
partition bases 0/32/64 for small-M matmuls).  mm1 runs M=32 with
zero-padded feature columns so the 16 garbage rows per 32-block are written
with clean zeros.  Partition-dim reductions (sum over hw) and broadcasts
(denom over hw) are done with tiny constant 0/1 matmuls (bd / exp3) fed
from host numpy.  The col-0 matmuls (denominator reduce, broadcast, mm2)
run as float32r (TF32, full PE rate); their operands are rounded to f32r by
the producing ACT/DVE ops.  mm1 stays fp32 (f32r cannot write PSUM at a
nonzero partition base).

32 batches per core = 10 groups of 3 + one ragged group [30, 31, 30] where
the duplicated slot's output is skipped.
"""

import numpy as np
from contextlib import ExitStack

import concourse.bass as bass
import concourse.bacc as bacc
import concourse.tile as tile
from concourse import mybir
from concourse.bass_utils import run_bass_kernel_spmd

F32 = mybir.dt.float32
F32R = mybir.dt.float32r
AF = mybir.ActivationFunctionType
ALU = mybir.AluOpType

N_CORES = 8
B_FULL, C, H, W, N = 256, 256, 4, 4, 2048
HW = H * W                  # 16
B = B_FULL // N_CORES       # 32 batches per core
KC = C // 128               # 2 contraction chunks of 128
GB = 3                      # batches per partition group (32-part offsets 0/32/64)
GP = 32 * GB                # 96 partitions used per group
NCH = 4                     # n chunks per group chain
NW = N // NCH               # 512 (one PSUM bank)
CONST = 1e-4


def make_groups(n_batch):
    """Chunks of GB batches; ragged tail padded with duplicates (emit=False)."""
    groups = []
    for s in range(0, n_batch, GB):
        real = list(range(s, min(s + GB, n_batch)))
        emit = [True] * len(real)
        while len(real) < GB:
            real.append(real[0])
            emit.append(False)
        groups.append((real, emit))
    return groups


def aux_inputs():
    # bd[k, m] = 1 iff row k is one of batch-slot m's real hw rows
    bd = np.zeros((GP, GB), np.float32)
    for k in range(GP):
        if k % 32 < HW:
            bd[k, k // 32] = 1.0
    # exp3[k, m] = 1 iff partition m belongs to batch-slot k's 32-block
    exp3 = np.zeros((GB, GP), np.float32)
    for m in range(GP):
        exp3[m // 32, m] = 1.0
    iden = np.eye(128, dtype=np.float32)
    return {"bd": bd, "exp3": exp3, "iden": iden}


def build_nc(n_batch=B, debug=False):
    nc = bacc.Bacc(None, target_bir_lowering=False, debug=debug)
    feat = nc.dram_tensor("fpad", [128, KC, n_batch, 32], F32, kind="ExternalInput")
    wts = nc.dram_tensor("weights", [n_batch, C, N], F32, kind="ExternalInput")
    out = nc.dram_tensor("out", [n_batch, C, N], F32, kind="ExternalOutput")
    bd_d = nc.dram_tensor("bd", [GP, GB], F32R, kind="ExternalInput")
    exp_d = nc.dram_tensor("exp3", [GB, GP], F32R, kind="ExternalInput")
    id_d = nc.dram_tensor("iden", [128, 128], F32, kind="ExternalInput")

    # [ci, b, kc, n] views of the DRAM tensors
    wts_r = wts.ap().rearrange("b (kc ci) n -> ci b kc n", kc=KC)
    out_r = out.ap().rearrange("b (kc ci) n -> ci b kc n", kc=KC)

    with tile.TileContext(nc) as tc, ExitStack() as ctx:
        singles = ctx.enter_context(tc.tile_pool(name="singles", bufs=1))
        wpool = ctx.enter_context(tc.tile_pool(name="w", bufs=5))
        opool = ctx.enter_context(tc.tile_pool(name="o", bufs=3))
        numpool = ctx.enter_context(tc.tile_pool(name="num", bufs=3))
        attpool = ctx.enter_context(tc.tile_pool(name="att", bufs=2))
        smallpool = ctx.enter_context(tc.tile_pool(name="small", bufs=3))
        ftpool = ctx.enter_context(tc.tile_pool(name="ft", bufs=2))
        ps_sc = ctx.enter_context(tc.tile_pool(name="ps_sc", bufs=4, space="PSUM"))
        ps_dr = ctx.enter_context(tc.tile_pool(name="ps_dr", bufs=1, space="PSUM"))
        ps_ft = ctx.enter_context(tc.tile_pool(name="ps_ft", bufs=1, space="PSUM"))
        ps_o = ctx.enter_context(tc.tile_pool(name="ps_o", bufs=2, space="PSUM"))

        bd_t = singles.tile([GP, GB], F32R)
        nc.sync.dma_start(out=bd_t, in_=bd_d.ap())
        exp_t = singles.tile([GB, GP], F32R)
        nc.sync.dma_start(out=exp_t, in_=exp_d.ap())
        id_t = singles.tile([128, 128], F32)
        nc.sync.dma_start(out=id_t, in_=id_d.ap())

        # features, pre-transposed + hw-padded to 32 with zeros on the host
        f_t = singles.tile([128, KC, n_batch, 32], F32)
        nc.sync.dma_start(out=f_t, in_=feat.ap())

        ev = 0
        for bs, emit in make_groups(n_batch):
            w_t = {}
            for b in set(bs):
                w_t[b] = wpool.tile([128, KC, N], F32, tag="w", name="w_t")
                nc.sync.dma_start(out=w_t[b], in_=wts_r[:, b])

            # transposed features fT[hw, c] for mm2.  Transposing the full
            # zero-padded [128, nreal, 32] slice puts slot j's fT at
            # partition 32j (transpose outputs must start at partition 0).
            nreal = len(set(bs))
            ft_ps = ps_ft.tile([32 * nreal, KC, 128], F32, name="ft_ps")
            for kc in range(KC):
                nc.tensor.transpose(
                    ft_ps[:, kc, :],
                    f_t[:, kc, bs[0] : bs[0] + nreal, :],
                    id_t,
                )
            ft_sb = ftpool.tile([32 * nreal, KC, 128], F32R, name="ft_sb")
            nc.scalar.copy(ft_sb, ft_ps)

            att_t = attpool.tile([GP, NCH, NW], F32R)
            # mm1 for all chunks first, then phase the ACT work (all Abs+Exp,
            # then all Lns) so the table-set switches happen twice per group
            # instead of twice per chunk; explicit deps pin the ACT order.
            sc_l, te_l, tl_l = [], [], []
            for nb in range(NCH):
                sc_ps = ps_sc.tile([GP, NW], F32, name="sc_ps")
                for j in range(GB):
                    for kc in range(KC):
                        nc.tensor.matmul(
                            sc_ps[32 * j : 32 * j + 32, :],
                            f_t[:, kc, bs[j], :],
                            w_t[bs[j]][:, kc, nb * NW : (nb + 1) * NW],
                            start=(kc == 0),
                            stop=(kc == KC - 1),
                        )
                sc_l.append(sc_ps)
            # softplus(x) = max(x,0) + ln(1 + exp(-|x|)): exp arg <= 0 so no
            # overflow, Ln input stays in [1,2]
            exp_insts = []
            for nb in range(NCH):
                t_abs = numpool.tile([GP, NW], F32, tag="tabs")
                nc.scalar.activation(t_abs, sc_l[nb], AF.Abs)
                t_exp = numpool.tile([GP, NW], F32, tag="texp", bufs=NCH)
                exp_insts.append(
                    nc.scalar.activation(t_exp, t_abs, AF.Exp, scale=-1.0)
                )
                te_l.append(t_exp)
            for nb in range(NCH):
                t_ln = numpool.tile([GP, NW], F32, tag="tln", bufs=NCH)
                ln_i = nc.scalar.activation(t_ln, te_l[nb], AF.Ln, bias=1.0)
                tile.add_dep_helper(
                    ln_i.ins, exp_insts[-1].ins, sync=False,
                    reason="cluster Lns after all Exps (one table switch)",
                )
                tl_l.append(t_ln)
            for nb in range(NCH):
                num_t = numpool.tile([GP, NW], F32R, tag="num")
                nc.vector.scalar_tensor_tensor(
                    num_t, sc_l[nb], 0.0, tl_l[nb], op0=ALU.max, op1=ALU.add
                )
                d_ps = ps_dr.tile([GB, NW], F32, tag="dr", name="d_ps")
                nc.tensor.matmul(
                    d_ps,
                    bd_t,
                    num_t,
                    start=True,
                    stop=True,
                )
                r_t = smallpool.tile([GB, NW], F32R)
                with nc.allow_low_precision(reason="tf32 matmul operand"):
                    nc.vector.tensor_scalar_add(r_t, d_ps, HW * CONST)
                    nc.vector.reciprocal(r_t, r_t)
                rb_ps = ps_dr.tile([GP, NW], F32, tag="dr", name="rb_ps")
                nc.tensor.matmul(
                    rb_ps,
                    exp_t,
                    r_t,
                    start=True,
                    stop=True,
                )
                # att = (num + CONST) * (1/denom)
                nc.vector.scalar_tensor_tensor(
                    att_t[:, nb, :],
                    num_t,
                    CONST,
                    rb_ps,
                    op0=ALU.add,
                    op1=ALU.mult,
                )

            for j in range(GB):
                if not emit[j]:
                    continue
                o_sb = opool.tile([128, KC, N], F32, tag="o", name="o_sb")
                for kc in range(KC):
                    for nb in range(NCH):
                        o_ps = ps_o.tile([128, NW], F32)
                        nc.tensor.matmul(
                            o_ps,
                            ft_sb[32 * j : 32 * j + HW, kc, :],
                            att_t[32 * j : 32 * j + HW, nb, :],
                            start=True,
                            stop=True,
                        )
                        dst = o_sb[:, kc, nb * NW : (nb + 1) * NW]
                        if ev % 2 == 0:
                            nc.vector.tensor_copy(dst, o_ps)
                        else:
                            nc.scalar.copy(dst, o_ps)
                        ev += 1
                nc.sync.dma_start(out=out_r[:, bs[j]], in_=o_sb)

    nc.compile()
    return nc


_NC_CACHE = {}


def _get_nc(n_batch=B):
    if n_batch not in _NC_CACHE:
        _NC_CACHE[n_batch] = build_nc(n_batch)
    return _NC_CACHE[n_batch]


def prep_features(features, dtype=np.float32):
    """[nb, C, H, W] f32 -> padded [128, KC, nb, 32] in dtype."""
    features = np.asarray(features).astype(dtype)
    nb = features.shape[0]
    f4 = features.reshape(nb, KC, 128, HW)
    fpad = np.zeros((nb, KC, 128, 32), dtype)
    fpad[..., :HW] = f4
    return np.ascontiguousarray(fpad.transpose(2, 1, 0, 3))  # [128, KC, nb, 32]


def run(features, weights, trace=False, **kwargs):
    """Shard over 8 cores, run, gather. Returns (out, BassKernelResults)."""
    fpad = prep_features(features)
    weights = np.ascontiguousarray(np.asarray(weights), dtype=np.float32)
    aux = aux_inputs()
    nc = _get_nc()
    in_maps = []
    for i in range(N_CORES):
        sl = slice(i * B, (i + 1) * B)
        in_maps.append(
            {"fpad": fpad[:, :, sl], "weights": weights[sl], **aux}
        )
    res = run_bass_kernel_spmd(
        nc, in_maps, core_ids=list(range(N_CORES)), trace=trace, **kwargs
    )
    out = np.concatenate([r["out"] for r in res.results], axis=0).astype(np.float32)
    return out, res


def kernel(features, weights):
    out, _ = run(features, weights)
    return out



# revision 7
# speedup vs baseline: 1.4925x; 1.4925x over previous
"""Attentional pooling layer on Trainium2 (Bass/Tile), 8-core batch-parallel.

Reference computation per batch b:
    scores[hw, n] = sum_c f[c, hw] * w[c, n]          (mm1, bf16 -> f32 PSUM)
    num           = softplus(scores)                  (ACT: single table op)
    denom[n]      = sum_hw num[hw, n] + 16*CONST      (PE reduce + DVE)
    att[hw, n]    = (num + CONST) / denom[n]          (PE bcast + DVE stt)
    out[c, n]     = sum_hw f[c, hw] * att[hw, n]      (mm2, bf16)

Memory-bound problem: per core 32 batches x (1 MiB weights in + 1 MiB out)
at bf16 ~= 64 MiB of HBM traffic -> ~186 us at the 360 GB/s DMA roofline.
All large tensors move as bf16 (inputs converted on host, output upcast on
host); accumulation stays f32 in PSUM.

Partition layout: 3 batches per 96-partition group at 32-partition offsets
(AP base partitions are restricted to 0/32/64).  mm1 runs M=32 with
zero-padded feature columns so pad rows get clean zeros.  Partition-dim
reduction (sum over hw) and broadcast (denom over hw) are tiny constant 0/1
matmuls (bd / exp3).  mm2's stationary fT comes pre-transposed from the
host.  Weight loads issue on the SP HWDGE queue, output stores on the ACT
HWDGE queue so neither head-blocks the other.  PSUM->SBUF output evictions
(the bf16 downcast) are split between ACT and DVE.

32 batches per core = 10 groups of 3 + one ragged group [30, 31, 30] where
the duplicated slot's mm2/store is skipped.
"""

import numpy as np
import ml_dtypes
from contextlib import ExitStack

import concourse.bass as bass
import concourse.bacc as bacc
import concourse.tile as tile
from concourse import mybir
from concourse.bass_utils import run_bass_kernel_spmd

F32 = mybir.dt.float32
BF16 = mybir.dt.bfloat16
AF = mybir.ActivationFunctionType
ALU = mybir.AluOpType
NP_BF16 = ml_dtypes.bfloat16

N_CORES = 8
B_FULL, C, H, W, N = 256, 256, 4, 4, 2048
HW = H * W                  # 16
B = B_FULL // N_CORES       # 32 batches per core
KC = C // 128               # 2 contraction chunks of 128
GB = 3                      # batches per partition group (32-part offsets)
GP = 32 * GB                # 96 partitions used per group
NCH = 4                     # n chunks per group chain
NW = N // NCH               # 512 (one PSUM bank)
CONST = 1e-4

# PSUM->SBUF output evictions per batch, round-robined over ACT/DVE/Pool so
# no single engine becomes the bottleneck (ACT also runs softplus, DVE the
# stt/recip work, Pool is otherwise idle).
EV_ENGINES = ("act", "dve", "act", "pool", "act", "dve", "act", "dve")


def make_groups(n_batch):
    """Chunks of GB batches; ragged tail padded with duplicates (emit=False)."""
    groups = []
    for s in range(0, n_batch, GB):
        real = list(range(s, min(s + GB, n_batch)))
        emit = [True] * len(real)
        while len(real) < GB:
            real.append(real[0])
            emit.append(False)
        groups.append((real, emit))
    return groups


def aux_inputs():
    # bd[k, m] = 1 iff partition k is one of batch-slot m's real hw rows
    bd = np.zeros((GP, GB), NP_BF16)
    for k in range(GP):
        if k % 32 < HW:
            bd[k, k // 32] = 1.0
    # exp3[m, p] = 1 iff partition p belongs to batch-slot m's 32-block
    exp3 = np.zeros((GB, GP), NP_BF16)
    for p in range(GP):
        exp3[p // 32, p] = 1.0
    return {"bd": bd, "exp3": exp3}


def build_nc(n_batch=B, debug=False):
    groups = make_groups(n_batch)
    ng = len(groups)
    nc = bacc.Bacc(None, target_bir_lowering=False, debug=debug)
    feat = nc.dram_tensor("fpad", [128, KC, n_batch, 32], BF16, kind="ExternalInput")
    ftr = nc.dram_tensor("ft", [GP, ng, KC, 128], BF16, kind="ExternalInput")
    wts = nc.dram_tensor("weights", [n_batch, C, N], BF16, kind="ExternalInput")
    out = nc.dram_tensor("out", [n_batch, C, N], BF16, kind="ExternalOutput")
    bd_d = nc.dram_tensor("bd", [GP, GB], BF16, kind="ExternalInput")
    exp_d = nc.dram_tensor("exp3", [GB, GP], BF16, kind="ExternalInput")

    # [ci, b, kc, n] views of the DRAM tensors
    wts_r = wts.ap().rearrange("b (kc ci) n -> ci b kc n", kc=KC)
    out_r = out.ap().rearrange("b (kc ci) n -> ci b kc n", kc=KC)

    # const AP for the Ln scale/bias that folds +CONST into softplus
    cs = float(np.exp(CONST))
    cs_t = nc.alloc_sbuf_tensor(f"const-float32-{cs}", [128, 1], F32)
    nc.gpsimd.memset(cs_t.ap(), cs)
    nc.const_aps.aps[(F32, cs)] = cs_t.ap()

    with tile.TileContext(nc) as tc, ExitStack() as ctx:
        singles = ctx.enter_context(tc.tile_pool(name="singles", bufs=1))
        wpool = ctx.enter_context(tc.tile_pool(name="w", bufs=8))
        opool = ctx.enter_context(tc.tile_pool(name="o", bufs=3))
        numpool = ctx.enter_context(tc.tile_pool(name="num", bufs=3))
        attpool = ctx.enter_context(tc.tile_pool(name="att", bufs=2))
        smallpool = ctx.enter_context(tc.tile_pool(name="small", bufs=3))
        ps_sc = ctx.enter_context(tc.tile_pool(name="ps_sc", bufs=3, space="PSUM"))
        ps_dr = ctx.enter_context(tc.tile_pool(name="ps_dr", bufs=2, space="PSUM"))
        ps_o = ctx.enter_context(tc.tile_pool(name="ps_o", bufs=3, space="PSUM"))

        bd_t = singles.tile([GP, GB], BF16)
        nc.sync.dma_start(out=bd_t, in_=bd_d.ap())
        exp_t = singles.tile([GB, GP], BF16)
        nc.sync.dma_start(out=exp_t, in_=exp_d.ap())

        # features: pre-transposed + hw-padded to 32 with zeros on the host
        f_t = singles.tile([128, KC, n_batch, 32], BF16)
        nc.sync.dma_start(out=f_t, in_=feat.ap())
        # fT[32*j+hw, g, kc, ci] for mm2's stationary operand
        ft_t = singles.tile([GP, ng, KC, 128], BF16)
        nc.sync.dma_start(out=ft_t, in_=ftr.ap())

        for g, (bs, emit) in enumerate(groups):
            w_t = {}
            for b in set(bs):
                w_t[b] = wpool.tile([128, KC, N], BF16, tag="w", name="w_t")
                nc.sync.dma_start(out=w_t[b], in_=wts_r[:, b])

            att_t = attpool.tile([GP, NCH, NW], BF16)
            for nb in range(NCH):
                sc_ps = ps_sc.tile([GP, NW], F32, name="sc_ps")
                for j in range(GB):
                    for kc in range(KC):
                        nc.tensor.matmul(
                            sc_ps[32 * j : 32 * j + 32, :],
                            f_t[:, kc, bs[j], :],
                            w_t[bs[j]][:, kc, nb * NW : (nb + 1) * NW],
                            start=(kc == 0),
                            stop=(kc == KC - 1),
                        )
                # softplus(x) + CONST = max(x,0) + ln((1+CONST')(1 + exp(-|x|)))
                # with ln(1+CONST') = CONST, folded into the Ln scale/bias.
                # numc = softplus(scores) + CONST; denom = sum_hw numc (the
                # 16*CONST rides along); att = numc / denom.
                t_abs = numpool.tile([GP, NW], F32, tag="tabs")
                nc.scalar.activation(t_abs, sc_ps, AF.Abs)
                t_exp = numpool.tile([GP, NW], F32, tag="texp")
                nc.scalar.activation(t_exp, t_abs, AF.Exp, scale=-1.0)
                t_ln = numpool.tile([GP, NW], F32, tag="tln")
                cs = float(np.exp(CONST))
                nc.scalar.activation(t_ln, t_exp, AF.Ln, scale=cs, bias=cs)
                num_t = numpool.tile([GP, NW], BF16, tag="num")
                with nc.allow_low_precision(reason="bf16 att numerator"):
                    nc.vector.scalar_tensor_tensor(
                        num_t, sc_ps, 0.0, t_ln, op0=ALU.max, op1=ALU.add
                    )
                d_ps = ps_dr.tile([GB, NW], F32, tag="dr", name="d_ps")
                nc.tensor.matmul(d_ps, bd_t, num_t, start=True, stop=True)
                r_t = smallpool.tile([GB, NW], BF16)
                with nc.allow_low_precision(reason="bf16 denom reciprocal"):
                    nc.vector.reciprocal(r_t, d_ps)
                rb_ps = ps_dr.tile([GP, NW], F32, tag="dr", name="rb_ps")
                nc.tensor.matmul(rb_ps, exp_t, r_t, start=True, stop=True)
                # att = numc * (1/denom)
                with nc.allow_low_precision(reason="bf16 att"):
                    nc.vector.tensor_tensor(
                        att_t[:, nb, :], num_t, rb_ps, op=ALU.mult
                    )

            for j in range(GB):
                if not emit[j]:
                    continue
                o_sb = opool.tile([128, KC, N], BF16, tag="o", name="o_sb")
                ev = 0
                for kc in range(KC):
                    for nb in range(NCH):
                        o_ps = ps_o.tile([128, NW], F32)
                        nc.tensor.matmul(
                            o_ps,
                            ft_t[32 * j : 32 * j + HW, g, kc, :],
                            att_t[32 * j : 32 * j + HW, nb, :],
                            start=True,
                            stop=True,
                        )
                        dst = o_sb[:, kc, nb * NW : (nb + 1) * NW]
                        eng = EV_ENGINES[ev]
                        if eng == "act":
                            nc.scalar.copy(dst, o_ps)
                        elif eng == "pool":
                            nc.gpsimd.tensor_copy(dst, o_ps)
                        else:
                            nc.vector.tensor_copy(dst, o_ps)
                        ev += 1
                nc.scalar.dma_start(out=out_r[:, bs[j]], in_=o_sb)

    nc.compile()
    return nc


_NC_CACHE = {}


def _get_nc(n_batch=B):
    if n_batch not in _NC_CACHE:
        _NC_CACHE[n_batch] = build_nc(n_batch)
    return _NC_CACHE[n_batch]


def prep_features(features):
    """[nb, C, H, W] f32 -> (fpad [128, KC, nb, 32],
    ft [n_cores, GP, ng, KC, 128])."""
    features = np.asarray(features, dtype=np.float32)
    nb = features.shape[0]
    f4 = features.reshape(nb, KC, 128, HW).astype(NP_BF16)
    fpad = np.zeros((nb, KC, 128, 32), NP_BF16)
    fpad[..., :HW] = f4
    fpad = np.ascontiguousarray(fpad.transpose(2, 1, 0, 3))  # [128, KC, nb, 32]

    groups = make_groups(B)
    ng = len(groups)
    ncores = nb // B
    ft = np.zeros((ncores, GP, ng, KC, 128), NP_BF16)
    for i in range(ncores):
        for g, (bs, emit) in enumerate(groups):
            for j, b in enumerate(bs):
                if not emit[j]:
                    continue
                # [KC, 128, HW] -> [HW, KC, 128]
                ft[i, 32 * j : 32 * j + HW, g] = f4[i * B + b].transpose(2, 0, 1)
    return fpad, ft


def run(features, weights, trace=False, **kwargs):
    """Shard over 8 cores, run, gather. Returns (out, BassKernelResults)."""
    fpad, ft = prep_features(features)
    weights = np.asarray(weights, dtype=np.float32).astype(NP_BF16)
    aux = aux_inputs()
    nc = _get_nc()
    in_maps = []
    for i in range(N_CORES):
        sl = slice(i * B, (i + 1) * B)
        in_maps.append(
            {"fpad": fpad[:, :, sl], "ft": ft[i], "weights": weights[sl], **aux}
        )
    res = run_bass_kernel_spmd(
        nc, in_maps, core_ids=list(range(N_CORES)), trace=trace, **kwargs
    )
    out = np.concatenate([r["out"] for r in res.results], axis=0).astype(np.float32)
    return out, res


def kernel(features, weights):
    out, _ = run(features, weights)
    return out


# revision 8
# speedup vs baseline: 1.7859x; 1.1966x over previous
"""Attentional pooling layer on Trainium2 (Bass/Tile), 8-core batch-parallel.

Reference computation per batch b:
    scores[hw, n] = sum_c f[c, hw] * w[c, n]          (mm1, bf16 -> f32 PSUM)
    num           = softplus(scores)                  (ACT: single table op)
    denom[n]      = sum_hw num[hw, n] + 16*CONST      (PE reduce + DVE)
    att[hw, n]    = (num + CONST) / denom[n]          (PE bcast + DVE stt)
    out[c, n]     = sum_hw f[c, hw] * att[hw, n]      (mm2, bf16)

Memory-bound problem: per core 32 batches x (1 MiB weights in + 1 MiB out)
at bf16 ~= 64 MiB of HBM traffic -> ~186 us at the 360 GB/s DMA roofline.
All large tensors move as bf16 (inputs converted on host, output upcast on
host); accumulation stays f32 in PSUM.

Partition layout: 3 batches per 96-partition group at 32-partition offsets
(AP base partitions are restricted to 0/32/64).  mm1 runs M=32 with
zero-padded feature columns so pad rows get clean zeros.  Partition-dim
reduction (sum over hw) and broadcast (denom over hw) are tiny constant 0/1
matmuls (bd / exp3).  mm2's stationary fT comes pre-transposed from the
host.  Weight loads issue on the SP HWDGE queue, output stores on the ACT
HWDGE queue so neither head-blocks the other.  PSUM->SBUF output evictions
(the bf16 downcast) are split between ACT and DVE.

32 batches per core = 10 groups of 3 + one ragged group [30, 31, 30] where
the duplicated slot's mm2/store is skipped.
"""

import numpy as np
import ml_dtypes
from contextlib import ExitStack

import concourse.bass as bass
import concourse.bacc as bacc
import concourse.tile as tile
from concourse import mybir
from concourse.bass_utils import run_bass_kernel_spmd

F32 = mybir.dt.float32
BF16 = mybir.dt.bfloat16
AF = mybir.ActivationFunctionType
ALU = mybir.AluOpType
NP_BF16 = ml_dtypes.bfloat16

N_CORES = 8
B_FULL, C, H, W, N = 256, 256, 4, 4, 2048
HW = H * W                  # 16
B = B_FULL // N_CORES       # 32 batches per core
KC = C // 128               # 2 contraction chunks of 128
GB = 3                      # batches per partition group (32-part offsets)
GP = 32 * GB                # 96 partitions used per group
NCH = 4                     # n chunks per group chain
NW = N // NCH               # 512 (one PSUM bank)
CONST = 1e-4

# PSUM->SBUF output evictions per batch, round-robined over ACT/DVE/Pool so
# no single engine becomes the bottleneck (ACT also runs softplus, DVE the
# stt/recip work, Pool is otherwise idle).
EV_ENGINES = ("act", "dve", "act", "pool", "act", "dve", "act", "dve")


def make_groups(n_batch):
    """Chunks of GB batches; ragged tail padded with duplicates (emit=False)."""
    groups = []
    for s in range(0, n_batch, GB):
        real = list(range(s, min(s + GB, n_batch)))
        emit = [True] * len(real)
        while len(real) < GB:
            real.append(real[0])
            emit.append(False)
        groups.append((real, emit))
    return groups


def aux_inputs():
    # bd[k, m] = 1 iff partition k is one of batch-slot m's real hw rows
    bd = np.zeros((GP, GB), NP_BF16)
    for k in range(GP):
        if k % 32 < HW:
            bd[k, k // 32] = 1.0
    # exp3[m, p] = 1 iff partition p belongs to batch-slot m's 32-block
    exp3 = np.zeros((GB, GP), NP_BF16)
    for p in range(GP):
        exp3[p // 32, p] = 1.0
    return {"bd": bd, "exp3": exp3}


def build_nc(n_batch=B, debug=False):
    groups = make_groups(n_batch)
    ng = len(groups)
    nc = bacc.Bacc(None, target_bir_lowering=False, debug=debug)
    feat = nc.dram_tensor("fpad", [128, KC, n_batch, 32], BF16, kind="ExternalInput")
    ftr = nc.dram_tensor("ft", [GP, ng, KC, 128], BF16, kind="ExternalInput")
    wts = nc.dram_tensor("weights", [n_batch, C, N], BF16, kind="ExternalInput")
    out = nc.dram_tensor("out", [n_batch, C, N], BF16, kind="ExternalOutput")
    bd_d = nc.dram_tensor("bd", [GP, GB], BF16, kind="ExternalInput")
    exp_d = nc.dram_tensor("exp3", [GB, GP], BF16, kind="ExternalInput")

    # [ci, b, kc, n] views of the DRAM tensors
    wts_r = wts.ap().rearrange("b (kc ci) n -> ci b kc n", kc=KC)
    out_r = out.ap().rearrange("b (kc ci) n -> ci b kc n", kc=KC)

    # const AP for the Ln scale/bias that folds +CONST into softplus
    cs = float(np.exp(CONST))
    cs_t = nc.alloc_sbuf_tensor(f"const-float32-{cs}", [128, 1], F32)
    nc.gpsimd.memset(cs_t.ap(), cs)
    nc.const_aps.aps[(F32, cs)] = cs_t.ap()

    with tile.TileContext(nc) as tc, ExitStack() as ctx:
        singles = ctx.enter_context(tc.tile_pool(name="singles", bufs=1))
        wpool = ctx.enter_context(tc.tile_pool(name="w", bufs=8))
        opool = ctx.enter_context(tc.tile_pool(name="o", bufs=3))
        numpool = ctx.enter_context(tc.tile_pool(name="num", bufs=3))
        attpool = ctx.enter_context(tc.tile_pool(name="att", bufs=2))
        smallpool = ctx.enter_context(tc.tile_pool(name="small", bufs=3))
        ps_sc = ctx.enter_context(tc.tile_pool(name="ps_sc", bufs=3, space="PSUM"))
        ps_dr = ctx.enter_context(tc.tile_pool(name="ps_dr", bufs=2, space="PSUM"))
        ps_o = ctx.enter_context(tc.tile_pool(name="ps_o", bufs=3, space="PSUM"))

        bd_t = singles.tile([GP, GB], BF16)
        nc.sync.dma_start(out=bd_t, in_=bd_d.ap())
        exp_t = singles.tile([GB, GP], BF16)
        nc.sync.dma_start(out=exp_t, in_=exp_d.ap())

        # features: pre-transposed + hw-padded to 32 with zeros on the host
        f_t = singles.tile([128, KC, n_batch, 32], BF16)
        nc.sync.dma_start(out=f_t, in_=feat.ap())
        # fT[32*j+hw, g, kc, ci] for mm2's stationary operand
        ft_t = singles.tile([GP, ng, KC, 128], BF16)
        nc.sync.dma_start(out=ft_t, in_=ftr.ap())

        for g, (bs, emit) in enumerate(groups):
            w_t = {}
            for b in set(bs):
                w_t[b] = wpool.tile([128, KC, N], BF16, tag="w", name="w_t")
                nc.sync.dma_start(out=w_t[b], in_=wts_r[:, b])

            att_t = attpool.tile([GP, NCH, NW], BF16)
            for nb in range(NCH):
                sc_ps = ps_sc.tile([GP, NW], F32, name="sc_ps")
                for j in range(GB):
                    for kc in range(KC):
                        nc.tensor.matmul(
                            sc_ps[32 * j : 32 * j + 32, :],
                            f_t[:, kc, bs[j], :],
                            w_t[bs[j]][:, kc, nb * NW : (nb + 1) * NW],
                            start=(kc == 0),
                            stop=(kc == KC - 1),
                        )
                # softplus(x) + CONST = max(x,0) + ln((1+CONST')(1 + exp(-|x|)))
                # with ln(1+CONST') = CONST, folded into the Ln scale/bias.
                # numc = softplus(scores) + CONST; denom = sum_hw numc (the
                # 16*CONST rides along); att = numc / denom.
                t_abs = numpool.tile([GP, NW], F32, tag="tabs")
                nc.scalar.activation(t_abs, sc_ps, AF.Abs)
                t_exp = numpool.tile([GP, NW], F32, tag="texp")
                nc.scalar.activation(t_exp, t_abs, AF.Exp, scale=-1.0)
                t_ln = numpool.tile([GP, NW], F32, tag="tln")
                cs = float(np.exp(CONST))
                nc.scalar.activation(t_ln, t_exp, AF.Ln, scale=cs, bias=cs)
                num_t = numpool.tile([GP, NW], BF16, tag="num")
                with nc.allow_low_precision(reason="bf16 att numerator"):
                    nc.vector.scalar_tensor_tensor(
                        num_t, sc_ps, 0.0, t_ln, op0=ALU.max, op1=ALU.add
                    )
                d_ps = ps_dr.tile([GB, NW], F32, tag="dr", name="d_ps")
                nc.tensor.matmul(d_ps, bd_t, num_t, start=True, stop=True)
                r_t = smallpool.tile([GB, NW], BF16)
                with nc.allow_low_precision(reason="bf16 denom reciprocal"):
                    nc.vector.reciprocal(r_t, d_ps)
                rb_ps = ps_dr.tile([GP, NW], F32, tag="dr", name="rb_ps")
                nc.tensor.matmul(rb_ps, exp_t, r_t, start=True, stop=True)
                # att = numc * (1/denom)
                with nc.allow_low_precision(reason="bf16 att"):
                    nc.vector.tensor_tensor(
                        att_t[:, nb, :], num_t, rb_ps, op=ALU.mult
                    )

            for j in range(GB):
                if not emit[j]:
                    continue
                o_sb = opool.tile([128, KC, N], BF16, tag="o", name="o_sb")
                ev = 0
                for kc in range(KC):
                    for nb in range(NCH):
                        o_ps = ps_o.tile([128, NW], F32)
                        nc.tensor.matmul(
                            o_ps,
                            ft_t[32 * j : 32 * j + HW, g, kc, :],
                            att_t[32 * j : 32 * j + HW, nb, :],
                            start=True,
                            stop=True,
                        )
                        dst = o_sb[:, kc, nb * NW : (nb + 1) * NW]
                        eng = EV_ENGINES[ev]
                        if eng == "act":
                            nc.scalar.copy(dst, o_ps)
                        elif eng == "pool":
                            nc.gpsimd.tensor_copy(dst, o_ps)
                        else:
                            nc.vector.tensor_copy(dst, o_ps)
                        ev += 1
                nc.scalar.dma_start(out=out_r[:, bs[j]], in_=o_sb)

    nc.compile()
    _dedupe_act_table_loads(nc)
    return nc


def _dedupe_act_table_loads(nc):
    """All ACT funcs used here (Abs/Exp/Ln/Copy) live in one table set, but
    the greedy placement pass flips between smaller sets, inserting a 1283 ns
    load per flip.  Rewrite the first load to the covering set and drop the
    rest (they carry no sync info)."""
    from concourse.hw_specs import get_activation_tables

    fn = nc.m.functions[0]
    used = {
        inst.func
        for b in fn.blocks
        for inst in b.instructions
        if isinstance(inst, mybir.InstActivation)
    }
    tables = list(get_activation_tables(nc.m.arch).items())
    target = next(
        i for i, (_, funcs) in enumerate(tables) if used <= funcs
    )
    first = True
    for b in fn.blocks:
        keep = []
        for inst in b.instructions:
            if isinstance(inst, mybir.InstLoadActFuncSet):
                if not first:
                    continue
                inst.act_func_set_id = target
                first = False
            keep.append(inst)
        b.instructions = keep


_NC_CACHE = {}


def _get_nc(n_batch=B):
    if n_batch not in _NC_CACHE:
        _NC_CACHE[n_batch] = build_nc(n_batch)
    return _NC_CACHE[n_batch]


def prep_features(features):
    """[nb, C, H, W] f32 -> (fpad [128, KC, nb, 32],
    ft [n_cores, GP, ng, KC, 128])."""
    features = np.asarray(features, dtype=np.float32)
    nb = features.shape[0]
    f4 = features.reshape(nb, KC, 128, HW).astype(NP_BF16)
    fpad = np.zeros((nb, KC, 128, 32), NP_BF16)
    fpad[..., :HW] = f4
    fpad = np.ascontiguousarray(fpad.transpose(2, 1, 0, 3))  # [128, KC, nb, 32]

    groups = make_groups(B)
    ng = len(groups)
    ncores = nb // B
    ft = np.zeros((ncores, GP, ng, KC, 128), NP_BF16)
    for i in range(ncores):
        for g, (bs, emit) in enumerate(groups):
            for j, b in enumerate(bs):
                if not emit[j]:
                    continue
                # [KC, 128, HW] -> [HW, KC, 128]
                ft[i, 32 * j : 32 * j + HW, g] = f4[i * B + b].transpose(2, 0, 1)
    return fpad, ft


def run(features, weights, trace=False, **kwargs):
    """Shard over 8 cores, run, gather. Returns (out, BassKernelResults)."""
    fpad, ft = prep_features(features)
    weights = np.asarray(weights, dtype=np.float32).astype(NP_BF16)
    aux = aux_inputs()
    nc = _get_nc()
    in_maps = []
    for i in range(N_CORES):
        sl = slice(i * B, (i + 1) * B)
        in_maps.append(
            {"fpad": fpad[:, :, sl], "ft": ft[i], "weights": weights[sl], **aux}
        )
    res = run_bass_kernel_spmd(
        nc, in_maps, core_ids=list(range(N_CORES)), trace=trace, **kwargs
    )
    out = np.concatenate([r["out"] for r in res.results], axis=0).astype(np.float32)
    return out, res


def kernel(features, weights):
    out, _ = run(features, weights)
    return out


# revision 10
# speedup vs baseline: 1.9670x; 1.1014x over previous
"""Attentional pooling layer on Trainium2 (Bass/Tile), 8-core batch-parallel.

Reference computation per batch b:
    scores[hw, n] = sum_c f[c, hw] * w[c, n]          (mm1, bf16 -> f32 PSUM)
    num           = softplus(scores)                  (ACT: single table op)
    denom[n]      = sum_hw num[hw, n] + 16*CONST      (PE reduce + DVE)
    att[hw, n]    = (num + CONST) / denom[n]          (PE bcast + DVE stt)
    out[c, n]     = sum_hw f[c, hw] * att[hw, n]      (mm2, bf16)

Memory-bound problem: per core 32 batches x (1 MiB weights in + 1 MiB out)
at bf16 ~= 64 MiB of HBM traffic -> ~186 us at the 360 GB/s DMA roofline.
All large tensors move as bf16 (inputs converted on host, output upcast on
host); accumulation stays f32 in PSUM.

Partition layout: 3 batches per 96-partition group at 32-partition offsets
(AP base partitions are restricted to 0/32/64).  mm1 runs M=32 with
zero-padded feature columns so pad rows get clean zeros.  Partition-dim
reduction (sum over hw) and broadcast (denom over hw) are tiny constant 0/1
matmuls (bd / exp3).  mm2's stationary fT comes pre-transposed from the
host.  Weight loads issue on the SP HWDGE queue, output stores on the ACT
HWDGE queue so neither head-blocks the other.  PSUM->SBUF output evictions
(the bf16 downcast) are split between ACT and DVE.

32 batches per core = 10 groups of 3 + one ragged group [30, 31, 30] where
the duplicated slot's mm2/store is skipped.
"""

import numpy as np
import ml_dtypes
from contextlib import ExitStack

import concourse.bass as bass
import concourse.bacc as bacc
import concourse.tile as tile
from concourse import mybir
from concourse.bass_utils import run_bass_kernel_spmd

F32 = mybir.dt.float32
BF16 = mybir.dt.bfloat16
AF = mybir.ActivationFunctionType
ALU = mybir.AluOpType
NP_BF16 = ml_dtypes.bfloat16

N_CORES = 8
B_FULL, C, H, W, N = 256, 256, 4, 4, 2048
HW = H * W                  # 16
B = B_FULL // N_CORES       # 32 batches per core
KC = C // 128               # 2 contraction chunks of 128
GB = 3                      # batches per partition group (32-part offsets)
GP = 32 * GB                # 96 partitions used per group
NCH = 4                     # n chunks per group chain
NW = N // NCH               # 512 (one PSUM bank)
CONST = 1e-4

# PSUM->SBUF output evictions per batch, round-robined over ACT/DVE/Pool so
# no single engine becomes the bottleneck (ACT also runs softplus, DVE the
# stt/recip work, Pool is otherwise idle).
EV_ENGINES = ("act", "dve", "act", "pool", "act", "dve", "act", "dve")


def make_groups(n_batch):
    """Chunks of GB batches; ragged tail padded with duplicates (emit=False)."""
    groups = []
    for s in range(0, n_batch, GB):
        real = list(range(s, min(s + GB, n_batch)))
        emit = [True] * len(real)
        while len(real) < GB:
            real.append(real[0])
            emit.append(False)
        groups.append((real, emit))
    return groups


def aux_inputs():
    # bd[k, m] = 1 iff partition k is one of batch-slot m's real hw rows
    bd = np.zeros((GP, GB), NP_BF16)
    for k in range(GP):
        if k % 32 < HW:
            bd[k, k // 32] = 1.0
    # exp3[m, p] = 1 iff partition p belongs to batch-slot m's 32-block
    exp3 = np.zeros((GB, GP), NP_BF16)
    for p in range(GP):
        exp3[p // 32, p] = 1.0
    return {"bd": bd, "exp3": exp3}


def build_nc(n_batch=B, debug=False):
    groups = make_groups(n_batch)
    ng = len(groups)
    nc = bacc.Bacc(None, target_bir_lowering=False, debug=debug)
    feat = nc.dram_tensor("fpad", [128, KC, n_batch, 32], BF16, kind="ExternalInput")
    ftr = nc.dram_tensor("ft", [GP, ng, KC, 128], BF16, kind="ExternalInput")
    wts = nc.dram_tensor("weights", [n_batch, C, N], BF16, kind="ExternalInput")
    out = nc.dram_tensor("out", [n_batch, C, N], BF16, kind="ExternalOutput")
    bd_d = nc.dram_tensor("bd", [GP, GB], BF16, kind="ExternalInput")
    exp_d = nc.dram_tensor("exp3", [GB, GP], BF16, kind="ExternalInput")

    # [ci, b, kc, n] views of the DRAM tensors
    wts_r = wts.ap().rearrange("b (kc ci) n -> ci b kc n", kc=KC)
    out_r = out.ap().rearrange("b (kc ci) n -> ci b kc n", kc=KC)

    # const AP for the Ln scale/bias that folds +CONST into softplus
    cs = float(np.exp(CONST))
    cs_t = nc.alloc_sbuf_tensor(f"const-float32-{cs}", [128, 1], F32)
    nc.gpsimd.memset(cs_t.ap(), cs)
    nc.const_aps.aps[(F32, cs)] = cs_t.ap()

    with tile.TileContext(nc) as tc, ExitStack() as ctx:
        singles = ctx.enter_context(tc.tile_pool(name="singles", bufs=1))
        wpool = ctx.enter_context(tc.tile_pool(name="w", bufs=8))
        opool = ctx.enter_context(tc.tile_pool(name="o", bufs=3))
        numpool = ctx.enter_context(tc.tile_pool(name="num", bufs=3))
        attpool = ctx.enter_context(tc.tile_pool(name="att", bufs=2))
        smallpool = ctx.enter_context(tc.tile_pool(name="small", bufs=3))
        ps_sc = ctx.enter_context(tc.tile_pool(name="ps_sc", bufs=3, space="PSUM"))
        ps_dr = ctx.enter_context(tc.tile_pool(name="ps_dr", bufs=2, space="PSUM"))
        ps_o = ctx.enter_context(tc.tile_pool(name="ps_o", bufs=3, space="PSUM"))

        bd_t = singles.tile([GP, GB], BF16)
        nc.sync.dma_start(out=bd_t, in_=bd_d.ap())
        exp_t = singles.tile([GB, GP], BF16)
        nc.sync.dma_start(out=exp_t, in_=exp_d.ap())

        # features: pre-transposed + hw-padded to 32 with zeros on the host
        f_t = singles.tile([128, KC, n_batch, 32], BF16)
        nc.sync.dma_start(out=f_t, in_=feat.ap())
        # fT[32*j+hw, g, kc, ci] for mm2's stationary operand
        ft_t = singles.tile([GP, ng, KC, 128], BF16)
        nc.sync.dma_start(out=ft_t, in_=ftr.ap())

        def emit_out(g, bs, emit, att_t):
            """mm2 + PSUM->SBUF bf16 eviction + store for one group."""
            for j in range(GB):
                if not emit[j]:
                    continue
                o_sb = opool.tile([128, KC, N], BF16, tag="o", name="o_sb")
                ev = 0
                for kc in range(KC):
                    for nb in range(NCH):
                        o_ps = ps_o.tile([128, NW], F32)
                        nc.tensor.matmul(
                            o_ps,
                            ft_t[32 * j : 32 * j + HW, g, kc, :],
                            att_t[32 * j : 32 * j + HW, nb, :],
                            start=True,
                            stop=True,
                        )
                        dst = o_sb[:, kc, nb * NW : (nb + 1) * NW]
                        eng = EV_ENGINES[ev]
                        if eng == "act":
                            nc.scalar.copy(dst, o_ps)
                        elif eng == "pool":
                            nc.gpsimd.tensor_copy(dst, o_ps)
                        else:
                            nc.vector.tensor_copy(dst, o_ps)
                        ev += 1
                nc.scalar.dma_start(out=out_r[:, bs[j]], in_=o_sb)

        pending = None  # (g, bs, emit, att_t) awaiting mm2/store, 1-group skew
        for g, (bs, emit) in enumerate(groups):
            w_t = {}
            for b in set(bs):
                w_t[b] = wpool.tile([128, KC, N], BF16, tag="w", name="w_t")
                nc.sync.dma_start(out=w_t[b], in_=wts_r[:, b])

            att_t = attpool.tile([GP, NCH, NW], BF16)
            for nb in range(NCH):
                sc_ps = ps_sc.tile([GP, NW], F32, name="sc_ps")
                for j in range(GB):
                    for kc in range(KC):
                        nc.tensor.matmul(
                            sc_ps[32 * j : 32 * j + 32, :],
                            f_t[:, kc, bs[j], :],
                            w_t[bs[j]][:, kc, nb * NW : (nb + 1) * NW],
                            start=(kc == 0),
                            stop=(kc == KC - 1),
                        )
                # softplus(x) + CONST = max(x,0) + ln((1+CONST')(1 + exp(-|x|)))
                # with ln(1+CONST') = CONST, folded into the Ln scale/bias.
                # numc = softplus(scores) + CONST; denom = sum_hw numc (the
                # 16*CONST rides along); att = numc / denom.
                t_abs = numpool.tile([GP, NW], F32, tag="tabs")
                nc.scalar.activation(t_abs, sc_ps, AF.Abs)
                t_exp = numpool.tile([GP, NW], F32, tag="texp")
                nc.scalar.activation(t_exp, t_abs, AF.Exp, scale=-1.0)
                t_ln = numpool.tile([GP, NW], F32, tag="tln")
                cs = float(np.exp(CONST))
                nc.scalar.activation(t_ln, t_exp, AF.Ln, scale=cs, bias=cs)
                num_t = numpool.tile([GP, NW], BF16, tag="num")
                with nc.allow_low_precision(reason="bf16 att numerator"):
                    nc.vector.scalar_tensor_tensor(
                        num_t, sc_ps, 0.0, t_ln, op0=ALU.max, op1=ALU.add
                    )
                d_ps = ps_dr.tile([GB, NW], F32, tag="dr", name="d_ps")
                nc.tensor.matmul(d_ps, bd_t, num_t, start=True, stop=True)
                r_t = smallpool.tile([GB, NW], BF16)
                with nc.allow_low_precision(reason="bf16 denom reciprocal"):
                    nc.vector.reciprocal(r_t, d_ps)
                rb_ps = ps_dr.tile([GP, NW], F32, tag="dr", name="rb_ps")
                nc.tensor.matmul(rb_ps, exp_t, r_t, start=True, stop=True)
                # att = numc * (1/denom)
                with nc.allow_low_precision(reason="bf16 att"):
                    nc.vector.tensor_tensor(
                        att_t[:, nb, :], num_t, rb_ps, op=ALU.mult
                    )

            if pending is not None:
                emit_out(*pending)
            pending = (g, bs, emit, att_t)
        emit_out(*pending)

    nc.compile()
    _dedupe_act_table_loads(nc)
    return nc


def _dedupe_act_table_loads(nc):
    """All ACT funcs used here (Abs/Exp/Ln/Copy) live in one table set, but
    the greedy placement pass flips between smaller sets, inserting a 1283 ns
    load per flip.  Rewrite the first load to the covering set and drop the
    rest (they carry no sync info)."""
    from concourse.hw_specs import get_activation_tables

    fn = nc.m.functions[0]
    used = {
        inst.func
        for b in fn.blocks
        for inst in b.instructions
        if isinstance(inst, mybir.InstActivation)
    }
    tables = list(get_activation_tables(nc.m.arch).items())
    target = next(
        i for i, (_, funcs) in enumerate(tables) if used <= funcs
    )
    first = True
    for b in fn.blocks:
        keep = []
        for inst in b.instructions:
            if isinstance(inst, mybir.InstLoadActFuncSet):
                if not first:
                    continue
                inst.act_func_set_id = target
                first = False
            keep.append(inst)
        b.instructions = keep


_NC_CACHE = {}


def _get_nc(n_batch=B):
    if n_batch not in _NC_CACHE:
        _NC_CACHE[n_batch] = build_nc(n_batch)
    return _NC_CACHE[n_batch]


def prep_features(features):
    """[nb, C, H, W] f32 -> (fpad [128, KC, nb, 32],
    ft [n_cores, GP, ng, KC, 128])."""
    features = np.asarray(features, dtype=np.float32)
    nb = features.shape[0]
    f4 = features.reshape(nb, KC, 128, HW).astype(NP_BF16)
    fpad = np.zeros((nb, KC, 128, 32), NP_BF16)
    fpad[..., :HW] = f4
    fpad = np.ascontiguousarray(fpad.transpose(2, 1, 0, 3))  # [128, KC, nb, 32]

    groups = make_groups(B)
    ng = len(groups)
    ncores = nb // B
    ft = np.zeros((ncores, GP, ng, KC, 128), NP_BF16)
    for i in range(ncores):
        for g, (bs, emit) in enumerate(groups):
            for j, b in enumerate(bs):
                if not emit[j]:
                    continue
                # [KC, 128, HW] -> [HW, KC, 128]
                ft[i, 32 * j : 32 * j + HW, g] = f4[i * B + b].transpose(2, 0, 1)
    return fpad, ft


def run(features, weights, trace=False, **kwargs):
    """Shard over 8 cores, run, gather. Returns (out, BassKernelResults)."""
    fpad, ft = prep_features(features)
    weights = np.asarray(weights, dtype=np.float32).astype(NP_BF16)
    aux = aux_inputs()
    nc = _get_nc()
    in_maps = []
    for i in range(N_CORES):
        sl = slice(i * B, (i + 1) * B)
        in_maps.append(
            {"fpad": fpad[:, :, sl], "ft": ft[i], "weights": weights[sl], **aux}
        )
    res = run_bass_kernel_spmd(
        nc, in_maps, core_ids=list(range(N_CORES)), trace=trace, **kwargs
    )
    out = np.concatenate([r["out"] for r in res.results], axis=0).astype(np.float32)
    return out, res


def kernel(features, weights):
    out, _ = run(features, weights)
    return out


# revision 24
# speedup vs baseline: 2.3562x; 1.1979x over previous
"""Attentional pooling layer on Trainium2 (Bass/Tile), 8-core batch-parallel.

Reference computation per batch b:
    scores[hw, n] = sum_c f[c, hw] * w[c, n]          (mm1, bf16 -> f32 PSUM)
    num           = softplus(scores)                  (ACT: single table op)
    denom[n]      = sum_hw num[hw, n] + 16*CONST      (PE reduce + DVE)
    att[hw, n]    = (num + CONST) / denom[n]          (PE bcast + DVE stt)
    out[c, n]     = sum_hw f[c, hw] * att[hw, n]      (mm2, bf16)

Memory-bound problem: per core 32 batches x (1 MiB weights in + 1 MiB out)
at bf16 ~= 64 MiB of HBM traffic -> ~186 us at the 360 GB/s DMA roofline.
All large tensors move as bf16 (inputs converted on host, output upcast on
host); accumulation stays f32 in PSUM.

Partition layout: 3 batches per 96-partition group at 32-partition offsets
(AP base partitions are restricted to 0/32/64).  mm1 runs M=32 with
zero-padded feature columns so pad rows get clean zeros.  Partition-dim
reduction (sum over hw) and broadcast (denom over hw) are tiny constant 0/1
matmuls (bd / exp3).  mm2's stationary fT comes pre-transposed from the
host.  Weight loads issue on the SP HWDGE queue, output stores on the ACT
HWDGE queue so neither head-blocks the other.  PSUM->SBUF output evictions
(the bf16 downcast) are split between ACT and DVE.

32 batches per core = 10 groups of 3 + one ragged group [30, 31, 30] where
the duplicated slot's mm2/store is skipped.
"""

import numpy as np
import ml_dtypes
from contextlib import ExitStack

import concourse.bass as bass
import concourse.bacc as bacc
import concourse.tile as tile
from concourse import mybir
from concourse.bass_utils import run_bass_kernel_spmd

F32 = mybir.dt.float32
BF16 = mybir.dt.bfloat16
AF = mybir.ActivationFunctionType
ALU = mybir.AluOpType
NP_BF16 = ml_dtypes.bfloat16

N_CORES = 8
B_FULL, C, H, W, N = 256, 256, 4, 4, 2048
HW = H * W                  # 16
B = B_FULL // N_CORES       # 32 batches per core
KC = C // 128               # 2 contraction chunks of 128
GB = 3                      # batches per partition group (32-part offsets)
GP = 32 * GB                # 96 partitions used per group
NCH = 4                     # n chunks per group chain
NW = N // NCH               # 512 (one PSUM bank)
CONST = 1e-4

# PSUM->SBUF output evictions per batch, round-robined over ACT/DVE/Pool so
# no single engine becomes the bottleneck (ACT also runs softplus, DVE the
# stt/recip work, Pool is otherwise idle).
EV_ENGINES = ("act", "dve", "act", "pool", "act", "dve", "act", "dve")


def make_groups(n_batch):
    """Chunks of GB batches; ragged tail padded with duplicates (emit=False)."""
    groups = []
    for s in range(0, n_batch, GB):
        real = list(range(s, min(s + GB, n_batch)))
        emit = [True] * len(real)
        while len(real) < GB:
            real.append(real[0])
            emit.append(False)
        groups.append((real, emit))
    return groups


def aux_inputs():
    # bd[k, m] = 1 iff partition k is one of batch-slot m's real hw rows
    bd = np.zeros((GP, GB), NP_BF16)
    for k in range(GP):
        if k % 32 < HW:
            bd[k, k // 32] = 1.0
    # exp3[m, p] = 1 iff partition p belongs to batch-slot m's 32-block
    exp3 = np.zeros((GB, GP), NP_BF16)
    for p in range(GP):
        exp3[p // 32, p] = 1.0
    return {"bd": bd, "exp3": exp3}


def build_nc(n_batch=B, debug=False, store_eng="pool", wbufs=6,
             ev_engines=EV_ENGINES, nch=NCH, sc_bufs=3, o_bufs=3, o_pool_bufs=4,
             store_split=2, out_pos=3):
    groups = make_groups(n_batch)
    ng = len(groups)
    nc = bacc.Bacc(None, target_bir_lowering=False, debug=debug)
    feat = nc.dram_tensor("fpad", [128, KC, n_batch, 32], BF16, kind="ExternalInput")
    ftr = nc.dram_tensor("ft", [GP, ng, KC, 128], BF16, kind="ExternalInput")
    wts = nc.dram_tensor("weights", [n_batch, C, N], BF16, kind="ExternalInput")
    out = nc.dram_tensor("out", [n_batch, C, N], BF16, kind="ExternalOutput")
    bd_d = nc.dram_tensor("bd", [GP, GB], BF16, kind="ExternalInput")
    exp_d = nc.dram_tensor("exp3", [GB, GP], BF16, kind="ExternalInput")

    # [ci, b, kc, n] views of the DRAM tensors
    wts_r = wts.ap().rearrange("b (kc ci) n -> ci b kc n", kc=KC)
    out_r = out.ap().rearrange("b (kc ci) n -> ci b kc n", kc=KC)

    # const AP for the Ln scale/bias that folds +CONST into softplus
    cs = float(np.exp(CONST))
    cs_t = nc.alloc_sbuf_tensor(f"const-float32-{cs}", [128, 1], F32)
    nc.gpsimd.memset(cs_t.ap(), cs)
    nc.const_aps.aps[(F32, cs)] = cs_t.ap()

    with tile.TileContext(nc) as tc, ExitStack() as ctx:
        singles = ctx.enter_context(tc.tile_pool(name="singles", bufs=1))
        wpool = ctx.enter_context(tc.tile_pool(name="w", bufs=wbufs))
        opool = ctx.enter_context(tc.tile_pool(name="o", bufs=o_pool_bufs))
        numpool = ctx.enter_context(tc.tile_pool(name="num", bufs=3))
        attpool = ctx.enter_context(tc.tile_pool(name="att", bufs=2))
        smallpool = ctx.enter_context(tc.tile_pool(name="small", bufs=3))
        ps_sc = ctx.enter_context(tc.tile_pool(name="ps_sc", bufs=sc_bufs, space="PSUM"))
        ps_dr = ctx.enter_context(tc.tile_pool(name="ps_dr", bufs=2, space="PSUM"))
        ps_o = ctx.enter_context(tc.tile_pool(name="ps_o", bufs=o_bufs, space="PSUM"))

        # features: pre-transposed + hw-padded to 32 with zeros on the host.
        # f_t is the only DMA that gates the first mm1; the other aux loads
        # are emitted after the first group's weight loads (needed later).
        f_t = singles.tile([128, KC, n_batch, 32], BF16)
        nc.sync.dma_start(out=f_t, in_=feat.ap())
        bd_t = singles.tile([GP, GB], BF16)
        exp_t = singles.tile([GB, GP], BF16)
        ft_t = singles.tile([GP, ng, KC, 128], BF16)

        def emit_aux_loads():
            nc.sync.dma_start(out=bd_t, in_=bd_d.ap())
            nc.sync.dma_start(out=exp_t, in_=exp_d.ap())
            # fT[32*j+hw, g, kc, ci] for mm2's stationary operand
            nc.sync.dma_start(out=ft_t, in_=ftr.ap())

        store = {"act": nc.scalar, "sp": nc.sync, "pool": nc.gpsimd}[store_eng]

        def emit_out(g, bs, emit, att_t):
            """mm2 + PSUM->SBUF bf16 eviction + store for one group."""
            nch = att_t.shape[1]
            nw = N // nch
            for j in range(GB):
                if not emit[j]:
                    continue
                o_sb = opool.tile([128, KC, N], BF16, tag="o", name="o_sb")
                ev = 0
                for kc in range(KC):
                    for nb in range(nch):
                        o_ps = ps_o.tile([128, nw], F32)
                        nc.tensor.matmul(
                            o_ps,
                            ft_t[32 * j : 32 * j + HW, g, kc, :],
                            att_t[32 * j : 32 * j + HW, nb, :],
                            start=True,
                            stop=True,
                        )
                        dst = o_sb[:, kc, nb * nw : (nb + 1) * nw]
                        eng = ev_engines[ev]
                        if eng == "act":
                            nc.scalar.copy(dst, o_ps)
                        elif eng == "pool":
                            nc.gpsimd.tensor_copy(dst, o_ps)
                        else:
                            nc.vector.tensor_copy(dst, o_ps)
                        ev += 1
                    if store_split == KC:
                        store.dma_start(
                            out=out_r[:, bs[j], kc], in_=o_sb[:, kc]
                        )
                if store_split == 1:
                    store.dma_start(out=out_r[:, bs[j]], in_=o_sb)

        def emit_chunk(bs, att_t, nb, nw):
            """mm1 + softplus + denom/recip/broadcast + att for one n-chunk."""
            sc_ps = ps_sc.tile([GP, nw], F32, name="sc_ps")
            for j in range(GB):
                for kc in range(KC):
                    nc.tensor.matmul(
                        sc_ps[32 * j : 32 * j + 32, :],
                        f_t[:, kc, bs[j], :],
                        w_t[bs[j]][:, kc, nb * nw : (nb + 1) * nw],
                        start=(kc == 0),
                        stop=(kc == KC - 1),
                    )
            # softplus(x) + CONST = max(x,0) + ln((1+CONST')(1 + exp(-|x|)))
            # with ln(1+CONST') = CONST, folded into the Ln scale/bias.
            # numc = softplus(scores) + CONST; denom = sum_hw numc (the
            # 16*CONST rides along); att = numc / denom.
            t_abs = numpool.tile([GP, nw], F32, tag="tabs")
            nc.scalar.activation(t_abs, sc_ps, AF.Abs)
            t_exp = numpool.tile([GP, nw], F32, tag="texp")
            nc.scalar.activation(t_exp, t_abs, AF.Exp, scale=-1.0)
            t_ln = numpool.tile([GP, nw], F32, tag="tln")
            nc.scalar.activation(t_ln, t_exp, AF.Ln, scale=cs, bias=cs)
            num_t = numpool.tile([GP, nw], BF16, tag="num")
            with nc.allow_low_precision(reason="bf16 att numerator"):
                nc.vector.scalar_tensor_tensor(
                    num_t, sc_ps, 0.0, t_ln, op0=ALU.max, op1=ALU.add
                )
            d_ps = ps_dr.tile([GB, nw], F32, tag="dr", name="d_ps")
            nc.tensor.matmul(d_ps, bd_t, num_t, start=True, stop=True)
            r_t = smallpool.tile([GB, nw], BF16)
            with nc.allow_low_precision(reason="bf16 denom reciprocal"):
                nc.vector.reciprocal(r_t, d_ps)
            rb_ps = ps_dr.tile([GP, nw], F32, tag="dr", name="rb_ps")
            nc.tensor.matmul(rb_ps, exp_t, r_t, start=True, stop=True)
            # att = numc * (1/denom)
            with nc.allow_low_precision(reason="bf16 att"):
                nc.vector.tensor_tensor(
                    att_t[:, nb, :], num_t, rb_ps, op=ALU.mult
                )

        pending = None  # (g, bs, emit, att_t) awaiting mm2/store, 1-group skew
        for g, (bs, emit) in enumerate(groups):
            w_t = {}
            for b in set(bs):
                w_t[b] = wpool.tile([128, KC, N], BF16, tag="w", name="w_t")
                nc.sync.dma_start(out=w_t[b], in_=wts_r[:, b])
            if g == 0:
                emit_aux_loads()

            nw = N // nch
            att_t = attpool.tile([GP, nch, nw], BF16)
            # Emit the previous group's output block mid-way through this
            # group's chunks: its mm2 inputs are long ready, so the PE slots
            # in the 24 mm2s while the softplus chains of the later chunks
            # are still in flight, and stores launch ~half a group earlier.
            for nb in range(out_pos):
                emit_chunk(bs, att_t, nb, nw)
            if pending is not None:
                emit_out(*pending)
            for nb in range(out_pos, nch):
                emit_chunk(bs, att_t, nb, nw)
            pending = (g, bs, emit, att_t)
        emit_out(*pending)

    nc.compile()
    _dedupe_act_table_loads(nc)
    return nc


def _dedupe_act_table_loads(nc):
    """All ACT funcs used here (Abs/Exp/Ln/Copy) live in one table set, but
    the greedy placement pass flips between smaller sets, inserting a 1283 ns
    load per flip.  Rewrite the first load to the covering set and drop the
    rest (they carry no sync info)."""
    from concourse.hw_specs import get_activation_tables

    fn = nc.m.functions[0]
    used = {
        inst.func
        for b in fn.blocks
        for inst in b.instructions
        if isinstance(inst, mybir.InstActivation)
    }
    tables = list(get_activation_tables(nc.m.arch).items())
    target = next(
        i for i, (_, funcs) in enumerate(tables) if used <= funcs
    )
    first = True
    for b in fn.blocks:
        keep = []
        for inst in b.instructions:
            if isinstance(inst, mybir.InstLoadActFuncSet):
                if not first:
                    continue
                inst.act_func_set_id = target
                first = False
            keep.append(inst)
        b.instructions = keep


_NC_CACHE = {}


def _get_nc(n_batch=B):
    if n_batch not in _NC_CACHE:
        _NC_CACHE[n_batch] = build_nc(n_batch)
    return _NC_CACHE[n_batch]


def prep_features(features):
    """[nb, C, H, W] f32 -> (fpad [128, KC, nb, 32],
    ft [n_cores, GP, ng, KC, 128])."""
    features = np.asarray(features, dtype=np.float32)
    nb = features.shape[0]
    f4 = features.reshape(nb, KC, 128, HW).astype(NP_BF16)
    fpad = np.zeros((nb, KC, 128, 32), NP_BF16)
    fpad[..., :HW] = f4
    fpad = np.ascontiguousarray(fpad.transpose(2, 1, 0, 3))  # [128, KC, nb, 32]

    groups = make_groups(B)
    ng = len(groups)
    ncores = nb // B
    ft = np.zeros((ncores, GP, ng, KC, 128), NP_BF16)
    for i in range(ncores):
        for g, (bs, emit) in enumerate(groups):
            for j, b in enumerate(bs):
                if not emit[j]:
                    continue
                # [KC, 128, HW] -> [HW, KC, 128]
                ft[i, 32 * j : 32 * j + HW, g] = f4[i * B + b].transpose(2, 0, 1)
    return fpad, ft


def run(features, weights, trace=False, **kwargs):
    """Shard over 8 cores, run, gather. Returns (out, BassKernelResults)."""
    fpad, ft = prep_features(features)
    weights = np.asarray(weights, dtype=np.float32).astype(NP_BF16)
    aux = aux_inputs()
    nc = _get_nc()
    in_maps = []
    for i in range(N_CORES):
        sl = slice(i * B, (i + 1) * B)
        in_maps.append(
            {"fpad": fpad[:, :, sl], "ft": ft[i], "weights": weights[sl], **aux}
        )
    res = run_bass_kernel_spmd(
        nc, in_maps, core_ids=list(range(N_CORES)), trace=trace, **kwargs
    )
    out = np.concatenate([r["out"] for r in res.results], axis=0).astype(np.float32)
    return out, res


def kernel(features, weights):
    out, _ = run(features, weights)
    return out
